# revision 27
# baseline (speedup 1.0000x reference)
"""Windowed multi-head attention (MixedAttentionProcessor) on 8 trn2 cores.

Problem (hardcoded): hidden_states [B=16, S=4096, C=512] fp32, 8x8 windows
over a 64x64 token grid, 8 heads of dim 64, qkv [512,1536] + proj [512,512].

Sharding: pure data-parallel over batch: 2 batches per core, no collectives.

Per-core plan (2 batches = 16 groups; group = (batch, hi) = 8 windows = 512
tokens = one contiguous 1 MiB HBM block):
  1. DMA x token-major, PE-transpose to channel-major xT [512c, 512t].
  2. qkT = Wqk.T @ xT (channel-major, fp32r, N=512 matmuls, W stationary).
     v   = x @ Wv    (token-major,  fp32r, xT chunks stationary).
  3. Per head-pair: scores via qT.T@kT into one PSUM bank
     [p=(hpar,t), f=(w,s)], exp on ACT (scale folded, no max needed --
     scores are O(+-5)), row-sum + normalize on DVE, PE-transpose p to
     [p=(wpar,s), f=(hpar,t)], PV matmuls (bf16) accumulate oT channel-major.
  4. y = oT.T @ Wp + b (fp32r, token-major) -> DMA out.
"""

import os

import numpy as np

import concourse.bass as bass
from concourse import bacc
import concourse.mybir as mybir
import concourse.tile as tile
from concourse.bass_utils import run_bass_kernel_spmd
from concourse.masks import make_identity

F32 = mybir.dt.float32
F32R = mybir.dt.float32r
BF16 = mybir.dt.bfloat16

B, S, C = 16, 4096, 512
NCORES = 8
BPC = B // NCORES          # batches per core
NH, HD, WS = 8, 64, 8
T = WS * WS                # 64 tokens per window
SCALE = HD ** -0.5
HGRID = 64                 # token grid is 64x64
NHI = HGRID // WS          # 8 window-rows -> 8 groups per batch
GT = 8 * T                 # 512 tokens per group (8 windows)

TRACE = False              # set by test.py for profiling runs
_CACHE = {}


def _emit_front(nc, tc, pools, consts, x_ap, b2, hi):
    (sing, sb_io, sb_xT, sb_qk, sb_v, sb_p, sb_pT, sb_oT, sb_y, sb_rs,
     ps_tr, ps_mm, ps_sc, ps_o) = pools
    (wqk_sb, wv_sb, wp_sb, qb_sb, vb_sb, pb_sb, id_f32, id_bf16,
     id_f32r) = consts

    # ---- 1. load x token-major, HBM-natural order: tile q = image rows
    # 2q..2q+2 of this window-row block; token index = r*64 + wi*8 + cc.
    base = hi * GT
    xtok = []
    for tcks in range(4):
        t_tile = sb_io.tile([128, C], F32, tag=f"xtok{tcks}")
        nc.sync.dma_start(
            out=t_tile,
            in_=x_ap[b2, base + 128 * tcks: base + 128 * (tcks + 1), :])
        xtok.append(t_tile)

    # ---- 2. transpose to channel-major xT[kc] [128 ch, 512 t]
    xT = []
    for kc in range(4):
        trp = ps_tr.tile([128, GT], F32, tag="tr")
        for tcks in range(4):
            nc.tensor.transpose(
                out=trp[:, 128 * tcks:128 * (tcks + 1)],
                in_=xtok[tcks][:, 128 * kc:128 * (kc + 1)],
                identity=id_f32,
            )
        xt = sb_xT.tile([128, GT], F32R, tag=f"xT{kc}")
        nc.scalar.copy(
            out=xt.rearrange("p (wi r cc) -> p r wi cc", wi=WS, r=WS),
            in_=trp.rearrange("p (r wi cc) -> p r wi cc", r=WS, wi=WS))
        xT.append(xt)

    # ---- 3a. qkT chunks m=0..7 (q heads then k heads), channel-major, bf16
    qkT = []
    for m in range(8):
        ps = ps_mm.tile([128, GT], F32, tag="mm")
        for kc in range(4):
            nc.tensor.matmul(
                out=ps,
                lhsT=wqk_sb[kc][:, 128 * m:128 * (m + 1)],
                rhs=xT[kc],
                start=(kc == 0), stop=(kc == 3),
            )
        qk = sb_qk.tile([128, GT], BF16, tag=f"qkT{m}")
        # + qkv bias (per out-channel = per partition)
        nc.vector.tensor_scalar_add(out=qk, in0=ps, scalar1=qb_sb[:, m:m + 1])
        qkT.append(qk)

    # ---- 3b. v token-major [128 t, 512 c'], bf16
    vsb = []
    for tcks in range(4):
        ps = ps_mm.tile([128, C], F32, tag="mm")
        for kc in range(4):
            nc.tensor.matmul(
                out=ps,
                lhsT=xT[kc][:, 128 * tcks:128 * (tcks + 1)],
                rhs=wv_sb[kc],
                start=(kc == 0), stop=(kc == 3),
            )
        v = sb_v.tile([128, C], BF16, tag=f"v{tcks}")
        nc.vector.tensor_add(out=v, in0=ps, in1=vb_sb)  # + v bias (free dim)
        vsb.append(v)

    return qkT, vsb


def _emit_back(nc, tc, pools, consts, out_ap, b2, hi, qkT, vsb):
    (sing, sb_io, sb_xT, sb_qk, sb_v, sb_p, sb_pT, sb_oT, sb_y, sb_rs,
     ps_tr, ps_mm, ps_sc, ps_o) = pools
    (wqk_sb, wv_sb, wp_sb, qb_sb, vb_sb, pb_sb, id_f32, id_bf16,
     id_f32r) = consts
    base = hi * GT

    # ---- 4. attention, one window-pair jj at a time.
    # scores bank: [p = 64*hpar + t, f = 128*j + 64*wpar + s]
    # o bank (token-major): [p = 64*wpar + t, f = 64*h + hd]
    osb = []
    for jj in range(4):
        sc = ps_sc.tile([128, GT], F32, tag="sc", name=f"sc{jj}")
        for h in range(NH):
            j, hpar = h // 2, h % 2
            sl = slice(64 * hpar, 64 * (hpar + 1))
            for wpar in range(2):
                w = 2 * jj + wpar
                wsl = slice(64 * w, 64 * (w + 1))
                fsl = slice(128 * j + 64 * wpar, 128 * j + 64 * (wpar + 1))
                nc.tensor.matmul(
                    out=sc[sl, fsl],
                    lhsT=qkT[j][sl, wsl],          # qT_h [hd, t]
                    rhs=qkT[4 + j][sl, wsl],       # kT_h [hd, s]
                    start=True, stop=True,
                    tile_position=(64 * hpar, 64 * hpar),
                )
        # softmax (no max subtraction needed: |scores*SCALE| <~ 6)
        p = sb_p.tile([128, GT], BF16, tag="p", name=f"p{jj}")
        nc.scalar.activation(out=p, in_=sc,
                             func=mybir.ActivationFunctionType.Exp,
                             scale=SCALE)
        rs = sb_rs.tile([128, 8], F32, tag="rs", name=f"rs{jj}")
        nc.vector.reduce_sum(
            out=rs.unsqueeze(2),
            in_=p.rearrange("p (g s) -> p g s", s=T),
            axis=mybir.AxisListType.X,
        )
        rr = sb_rs.tile([128, 8], F32, tag="rr", name=f"rr{jj}")
        nc.vector.reciprocal(out=rr, in_=rs)
        nc.vector.tensor_mul(
            out=p.rearrange("p (g s) -> p g s", s=T),
            in0=p.rearrange("p (g s) -> p g s", s=T),
            in1=rr.unsqueeze(2).broadcast_to([128, 8, T]),
        )
        # transpose p -> pT [p = (wpar, s), f = 128*j + 64*hpar + t]
        trp = ps_tr.tile([128, GT], BF16, tag="tr", name=f"trp{jj}")
        for j in range(4):
            bsl = slice(128 * j, 128 * (j + 1))
            nc.tensor.transpose(out=trp[:, bsl], in_=p[:, bsl],
                                identity=id_bf16)
        pT = sb_pT.tile([128, GT], BF16, tag="pT", name=f"pT{jj}")
        nc.scalar.copy(out=pT, in_=trp)
        # PV into this pair's token-major o bank
        o_ps = ps_o.tile([128, C], F32, tag="o", name=f"o_ps{jj}")
        for h in range(NH):
            j, hpar = h // 2, h % 2
            for wpar in range(2):
                wrow = slice(64 * wpar, 64 * (wpar + 1))
                nc.tensor.matmul(
                    out=o_ps[wrow, 64 * h:64 * (h + 1)],
                    lhsT=pT[wrow, 128 * j + 64 * hpar:
                            128 * j + 64 * (hpar + 1)],          # pT [s, t]
                    rhs=vsb[jj][wrow, 64 * h:64 * (h + 1)],      # v [s, hd]
                    start=True, stop=True,
                    tile_position=(64 * wpar, 64 * wpar),
                )
        o = sb_p.tile([128, C], BF16, tag=f"osb{jj}", name=f"osb{jj}")
        nc.scalar.copy(out=o, in_=o_ps)
        osb.append(o)
    oT = []
    for kc in range(4):
        trp = ps_tr.tile([128, GT], BF16, tag="tr", name=f"trpo{kc}")
        for jj in range(4):
            nc.tensor.transpose(
                out=trp[:, 128 * jj:128 * (jj + 1)],
                in_=osb[jj][:, 128 * kc:128 * (kc + 1)],
                identity=id_bf16,
            )
        o = sb_oT.tile([128, GT], BF16, tag=f"oT{kc}", name=f"oTs{kc}")
        # trp free order is window-major (w, t); store natural (r, wi, cc)
        nc.scalar.copy(
            out=o.rearrange("p (r wi cc) -> p wi r cc", r=WS, wi=WS),
            in_=trp.rearrange("p (wi r cc) -> p wi r cc", wi=WS, r=WS))
        oT.append(o)

    # ---- 5. proj: y token-major [128 t, 512 c] = oT.T @ Wp + b
    # lhsT slice enumerates (r-pair, wi, cc) so y partitions come out in
    # HBM-natural order -> single contiguous DMA per tile.
    for tcks in range(4):
        ps = ps_mm.tile([128, C], F32, tag="mm")
        for kc in range(4):
            nc.tensor.matmul(
                out=ps,
                lhsT=oT[kc][:, 128 * tcks:128 * (tcks + 1)],
                rhs=wp_sb[kc],
                start=(kc == 0), stop=(kc == 3),
            )
        y = sb_y.tile([128, C], F32, tag="y")
        nc.vector.tensor_add(out=y, in0=ps, in1=pb_sb)
        nc.sync.dma_start(
            out=out_ap[b2, base + 128 * tcks: base + 128 * (tcks + 1), :],
            in_=y)


def _build_nc(groups=None, repeat=1):
    nc = bacc.Bacc()
    x_ap = nc.declare_dram_parameter("x", [BPC, S, C], F32, isOutput=False)
    qkv_w = nc.declare_dram_parameter("qkv_w", [C, 3 * C], F32R, isOutput=False)
    qkv_b = nc.declare_dram_parameter("qkv_b", [3 * C], F32, isOutput=False)
    proj_w = nc.declare_dram_parameter("proj_w", [C, C], F32, isOutput=False)
    proj_b = nc.declare_dram_parameter("proj_b", [C], F32, isOutput=False)
    out_ap = nc.declare_dram_parameter("out", [BPC, S, C], F32, isOutput=True)

    with tile.TileContext(nc) as tc:
        with (
            tc.tile_pool(name="sing", bufs=1) as sing,
            tc.tile_pool(name="sb_io", bufs=2) as sb_io,
            tc.tile_pool(name="sb_xT", bufs=2) as sb_xT,
            tc.tile_pool(name="sb_qk", bufs=2) as sb_qk,
            tc.tile_pool(name="sb_v", bufs=2) as sb_v,
            tc.tile_pool(name="sb_p", bufs=2) as sb_p,
            tc.tile_pool(name="sb_pT", bufs=2) as sb_pT,
            tc.tile_pool(name="sb_oT", bufs=2) as sb_oT,
            tc.tile_pool(name="sb_y", bufs=2) as sb_y,
            tc.tile_pool(name="sb_rs", bufs=2) as sb_rs,
            tc.tile_pool(name="ps_tr", bufs=2, space="PSUM") as ps_tr,
            tc.tile_pool(name="ps_mm", bufs=2, space="PSUM") as ps_mm,
            tc.tile_pool(name="ps_sc", bufs=2, space="PSUM") as ps_sc,
            tc.tile_pool(name="ps_o", bufs=2, space="PSUM") as ps_o,
        ):
            # --- constants: weights (channel-chunked), biases, identities
            wqk_sb, wv_sb, wp_sb = [], [], []
            for kc in range(4):
                ksl = slice(128 * kc, 128 * (kc + 1))
                wqk = sing.tile([128, 2 * C], F32R, tag=f"wqk{kc}")
                nc.sync.dma_start(out=wqk, in_=qkv_w[ksl, 0:2 * C])
                wqk_sb.append(wqk)
                wv = sing.tile([128, C], F32R, tag=f"wv{kc}")
                nc.sync.dma_start(out=wv, in_=qkv_w[ksl, 2 * C:3 * C])
                wv_sb.append(wv)
                wpf = sing.tile([128, C], F32, tag=f"wpf{kc}", name=f"wpf{kc}")
                nc.sync.dma_start(out=wpf, in_=proj_w[ksl, :])
                wp = sing.tile([128, C], BF16, tag=f"wp{kc}", name=f"wp{kc}")
                nc.vector.tensor_copy(out=wp, in_=wpf)
                wp_sb.append(wp)
            qb_sb = sing.tile([128, 8], F32, tag="qb")  # qk bias, per-partition
            nc.gpsimd.dma_start(
                out=qb_sb, in_=qkv_b.rearrange("(m p) -> p m", p=128)[:, 0:8])
            vb_sb = sing.tile([128, C], F32, tag="vb")  # v bias, bcast rows
            vb_src = qkv_b[2 * C:3 * C]
            nc.gpsimd.dma_start(
                out=vb_sb,
                in_=bass.AP(tensor=vb_src.tensor, offset=vb_src.offset,
                            ap=[[0, 128]] + [list(d) for d in vb_src.ap]))
            pb_sb = sing.tile([128, C], F32, tag="pb")  # proj bias, bcast rows
            pb_src = proj_b[:]
            nc.gpsimd.dma_start(
                out=pb_sb,
                in_=bass.AP(tensor=pb_src.tensor, offset=pb_src.offset,
                            ap=[[0, 128]] + [list(d) for d in pb_src.ap]))
            id_f32 = sing.tile([128, 128], F32, tag="idf")
            make_identity(nc, id_f32)
            id_bf16 = sing.tile([128, 128], BF16, tag="idb")
            make_identity(nc, id_bf16)
            id_f32r = sing.tile([128, 128], F32R, tag="idr")
            nc.vector.tensor_copy(out=id_f32r, in_=id_f32)

            consts = (wqk_sb, wv_sb, wp_sb, qb_sb, vb_sb, pb_sb,
                      id_f32, id_bf16, id_f32r)
            pools = (sing, sb_io, sb_xT, sb_qk, sb_v, sb_p, sb_pT, sb_oT,
                     sb_y, sb_rs, ps_tr, ps_mm, ps_sc, ps_o)
            if groups is None:
                groups = [(b2, hi) for b2 in range(BPC) for hi in range(NHI)]

            def _emit_all():
                pend = None
                for b2, hi in groups:
                    front = _emit_front(nc, tc, pools, consts, x_ap, b2, hi)
                    if pend is not None:
                        _emit_back(nc, tc, pools, consts, out_ap, *pend)
                    pend = (b2, hi, *front)
                _emit_back(nc, tc, pools, consts, out_ap, *pend)

            if repeat == 1:
                _emit_all()
            else:
                with tc.For_i(0, repeat, 1):
                    _emit_all()
    if not nc.is_finalized():
        nc.finalize()
    return nc


def kernel(attn_output=None, hidden_states=None, qkv_w=None, qkv_b=None,
           proj_w=None, proj_b=None):
    hs = np.ascontiguousarray(np.asarray(hidden_states, dtype=np.float32))
    qw = np.ascontiguousarray(np.asarray(qkv_w, dtype=np.float32))
    qb = np.ascontiguousarray(np.asarray(qkv_b, dtype=np.float32))
    pw = np.ascontiguousarray(np.asarray(proj_w, dtype=np.float32))
    pb = np.ascontiguousarray(np.asarray(proj_b, dtype=np.float32))

    if "nc" not in _CACHE:
        _CACHE["nc"] = _build_nc()
    nc = _CACHE["nc"]

    in_maps = [
        {"x": hs[BPC * i:BPC * (i + 1)], "qkv_w": qw, "qkv_b": qb,
         "proj_w": pw, "proj_b": pb}
        for i in range(NCORES)
    ]
    res = run_bass_kernel_spmd(nc, in_maps, list(range(NCORES)), trace=TRACE)
    if TRACE and res.exec_time_ns is not None:
        print(f"HW exec time: {res.exec_time_ns} ns")
    out = np.concatenate([res.results[i]["out"] for i in range(NCORES)],
                         axis=0)
    return out.reshape(B, S, C)
